# revision 55
# baseline (speedup 1.0000x reference)
"""Trainium2 Bass kernel for the pairwise concordance-index loss.

reference:
    loss = sum_{i<j, f_i=f_j=1} relu((p_i-p_j)(t_i-t_j)) / 100 / n_pairs

Math:
  Only flagged (f=1) entries contribute, so the host first COMPACTS the
  arrays to the n1 flagged entries (padded with zeros to NB*128), which
  shrinks the pairwise matrix from B^2 to ~(0.7B)^2.
  M[i,j] = (p_i-p_j)(t_i-t_j) = A^T B, rank 4:
      A = [u, 1, p, t],  B = [1, u, -t, -p],  u = p*t   (zeros in padding)
  sum relu(M) = 0.5*(sum M + sum |M|); sum M has an O(n) closed form done
  on the host in fp64; sum |M| is the O(n^2) part done on device.

Device decomposition (8 cores, identical program, data-sharded):
  The cyclic pair-cover ring has NB row-blocks of 128 rows, NB ODD and
  minimal (surplus block-SLOTS beyond the ring get all-zero A strips
  and contribute nothing).  For an odd ring, offsets e=1..(NB-1)/2
  cover every unordered block pair exactly once at full weight — no
  antipodal fold, no half-weight columns.  The e=0 diagonal blocks
  (~2% of pairs) are summed on the host in fp64.  Core k owns NBC
  block-slots as a quad gang (4) + duo gang (NBC-4).

Device structure (raw Bass, hand-rolled semaphores — no TileContext):
  PE: per 2-bank PSUM tile, K=4 bf16 matmuls packed into disjoint
  32-row PE groups via tile_position (concurrent matmuls always target
  distinct PSUM banks).  Each tile is consumed by ONE fused abs-row-sum
  job on the DVE (tensor_reduce(apply_absolute_value)) or the ScalarE
  (activation(Abs, accum_out)), greedily load-balanced (measured
  effective periods ~1131ns vs ~1181+339ns per 1024 elements).  PSUM
  is an 8-bank ring of four 2-bank buffers; the PE waits on the
  consumer semaphore before reusing a buffer.  The very first slot is
  emitted as two single-bank tiles so the PE can start on the minimum
  set of input chunks.

  Input DMAs: per-chunk semaphores (completions interleave across a
  queue's DMA engines, so cumulative counts are NOT safe).  The 'a'
  (weights) transfer is split per gang; a0 and b0h1 lead the sync
  queue while b1h1 leads the scalar queue, so tile 0's three chunks
  arrive on two queues in parallel.  Tiles are issued in data-arrival
  order.

  The output DMA is issued from the otherwise-idle sync engine, gated
  on both consumer sems (the A-side then_inc lands on
  ACTIVATION_READ_ACCUMULATOR, i.e. after the accumulator write), and
  its completion is NOT waited on: the NRT-injected postamble that
  follows (a ~7us storm resetting all 256 semaphores, engine by
  engine) far outlasts the DMA drain, so the store lands well before
  the runtime reads outputs.  Set KEEP_OUT_WAIT=1 to restore the
  explicit completion wait.
"""

import numpy as np

B = 8192
P = 128
NCORE = 8
CUT = 2560

_cache = {}


def _plan(n1):
    """Compile-time plan derived from the flagged count.

    The cyclic pair-cover ring has nb blocks with nb ODD and minimal: for
    an odd ring, offsets e=1..(nb-1)/2 cover every unordered block pair
    exactly once (pair distance d>eh is covered by the other endpoint at
    e=nb-d), so there is no antipodal fold and no half-weight columns.
    The e=0 diagonal blocks are summed on the host.  Each core runs nbc
    block-SLOTS (identical SPMD program); nbc*NCORE may exceed nb — the
    surplus slots get all-zero A strips and contribute nothing.
    """
    nb = max(3, -(-n1 // P))        # 128-row blocks needed
    nb += 1 - (nb & 1)              # odd ring
    nbc = -(-nb // NCORE)           # block-slots per core
    eh = (nb - 1) // 2              # max offset (full weight)
    mainw = P * (nbc - 1 + eh)      # shared slab for e=1..eh
    gangs = [4] * (nbc // 4) + ([nbc % 4] if nbc % 4 else [])
    w = P * eh                      # main cols per block
    nfull, rem = w // 512, w % 512
    return dict(nb=nb, nbc=nbc, eh=eh, mainw=mainw,
                bcols=mainw, gangs=gangs, nfull=nfull, rem=rem)


# DMA queue orders.  Completions within a HWDGE queue interleave across
# physical DMA engines, so each chunk gets its OWN semaphore (wait >= 16
# means that chunk fully landed); the order below only sets arrival time.
# 'a0' (first gang's weights) and 'b0h1a' (tile 0's columns, small) lead
# parallel queues so the first matmul can start as early as possible.
_SPLIT0 = 1536
_SYNC_ORDER = ["a0", "b0h1a", "b0h1b", "a0b", "a1", "b2h1", "b0h2", "b2h2",
               "b3h2"]
_SCAL_ORDER = ["b1h1", "b3h1", "b1h2"]


def _b_ranges(bcols):
    """chunk name -> (replica, lo, hi) for the B column transfers."""
    r = {}
    for q in range(4):
        h1 = [(f"b{q}h1", q, 0, CUT)]
        if q == 0:
            h1 = [("b0h1a", 0, 0, _SPLIT0), ("b0h1b", 0, _SPLIT0, CUT)]
        for name, qq, lo, hi in h1:
            r[name] = (qq, lo, hi)
        r[f"b{q}h2"] = (q, CUT, bcols)
    return r


def _chunk_info():
    info = {}
    for i, c in enumerate(_SYNC_ORDER):
        info[c] = ("s", i + 0.0)
    for i, c in enumerate(_SCAL_ORDER):
        info[c] = ("c", i + 0.5)
    return info


def _layout(plan):
    """Ordered tile descriptors: matmul lists, reduce specs, DMA gates."""
    gangs, rem, mainw = plan["gangs"], plan["rem"], plan["mainw"]
    cinfo = _chunk_info()
    tiles = []
    for g, sz in enumerate(gangs):
        off = sum(gangs[:g])
        npair = (sz + 1) // 2
        for pi in range(npair):
            qs = [q for q in (2 * pi, 2 * pi + 1) if q < sz]
            for s in range(plan["nfull"]):
                if g == 0 and pi == 0 and s == 0:
                    # very first slot as two single-bank tiles: tile 0 only
                    # needs the sync queue's first two chunks, so the PE
                    # starts ~0.7us earlier; consumers are idle then, so
                    # the extra small job costs no makespan
                    for q in qs:
                        mms = [(q, P * (off + q), 512, 0, 0)]
                        tiles.append(dict(acol=P * g, mms=mms, kind="main"))
                    continue
                mms = [(q, P * (off + q) + 512 * s, 512, qi, 0)
                       for qi, q in enumerate(qs)]
                tiles.append(dict(acol=P * g, mms=mms, kind="main"))
    # remainder tiles: the last rem columns of each block's main span
    if rem:
        for g, sz in enumerate(gangs):
            off = sum(gangs[:g])
            for pi in range((sz + 1) // 2):
                qs = [q for q in (2 * pi, 2 * pi + 1) if q < sz]
                mms = [(q, P * (off + q) + 512 * plan["nfull"], rem, qi, 0)
                       for qi, q in enumerate(qs)]
                tiles.append(dict(acol=P * g, mms=mms, kind="rem"))
    # per-tile chunk requirements -> (queue, threshold); arrival rank
    branges = _b_ranges(plan["bcols"])
    for t in tiles:
        # gang-0 weights arrive in two pieces: a0 = partitions 0-35 (q0/q1
        # strips, 36 descriptors — fast), a0b = partitions 36-127 (q2/q3)
        if t["acol"] == 0:
            need = {"a0" if q < 2 else "a0b" for (q, *_r) in t["mms"]}
        else:
            need = {"a1"}
        for (q, coff, n, _b, _c) in t["mms"]:
            for name, (qq, lo, hi) in branges.items():
                if qq == q and coff < hi and coff + n > lo:
                    need.add(name)
        t["nbank"] = len(set(b for (_, _, _, b, _) in t["mms"]))
        t["width"] = max(c + n for (_, _, n, _, c) in t["mms"])
        t["fd"] = sum(n for (_, _, n, _, _) in t["mms"])
        t["chunks"] = need
        t["rank"] = max(cinfo[c][1] for c in need)
    # issue order: by data arrival, stable within rank
    tiles.sort(key=lambda t: t["rank"])
    # consumer assignment: greedy balance of projected finish times using
    # HW-measured effective periods (V 1131ns @ fd=1024, A 1181ns), then
    # pairwise V<->A swaps (count-preserving, so the ring alternation
    # pattern is kept) to close any residual finish-time gap
    def cv(fd):
        return (fd + 62) / 0.96

    def ca(fd):
        return fd / 1.2 + 328.0

    tv = ta = 0.0
    for t in tiles:
        if tv + cv(t["fd"]) <= ta + ca(t["fd"]):
            t["eng"] = "V"
            tv += cv(t["fd"])
        else:
            t["eng"] = "A"
            ta += ca(t["fd"])

    def makespan():
        return max(sum(cv(t["fd"]) for t in tiles if t["eng"] == "V"),
                   sum(ca(t["fd"]) for t in tiles if t["eng"] == "A"))

    best = makespan()
    improved = True
    while improved:
        improved = False
        for t1 in tiles:
            for t2 in tiles:
                if t1["eng"] == "V" and t2["eng"] == "A":
                    t1["eng"], t2["eng"] = "A", "V"
                    m = makespan()
                    if m < best - 1e-9:
                        best = m
                        improved = True
                    else:
                        t1["eng"], t2["eng"] = "V", "A"
    return tiles


def _build(plan):
    """Build + compile the raw Bass module (once per plan)."""
    import os
    import concourse.bacc as bacc
    import concourse.mybir as mybir

    f32 = mybir.dt.float32
    bf16 = mybir.dt.bfloat16
    nc = bacc.Bacc("TRN2", target_bir_lowering=False, debug=False,
                   num_devices=NCORE)

    gangs = plan["gangs"]
    bcols = plan["bcols"]
    awidth = P * len(gangs)
    tiles = _layout(plan)
    njobs = len(tiles)
    nV = sum(1 for t in tiles if t["eng"] == "V")
    nA = njobs - nV

    a_dram = nc.dram_tensor("a_rows", [P, awidth], bf16, kind="ExternalInput")
    b_dram = nc.dram_tensor("b_cols", [4, bcols], bf16, kind="ExternalInput")
    acc_dram = nc.dram_tensor("acc", [P, njobs], f32, kind="ExternalOutput")

    a_sb = nc.alloc_sbuf_tensor("a_sb", [P, awidth], bf16)
    b_sb = nc.alloc_sbuf_tensor("b_sb", [P, bcols], bf16)
    acc_sb = nc.alloc_sbuf_tensor("acc_sb", [P, njobs], f32)
    ps = nc.alloc_psum_tensor("ps", [P, 8, 512], f32)

    sem_mm = nc.alloc_semaphore("sem_mm")    # tiles filled by PE
    sem_v = nc.alloc_semaphore("sem_v")      # DVE jobs done
    sem_a = nc.alloc_semaphore("sem_a")      # ACT jobs done
    # one sem per input chunk: completions interleave within a queue
    chunk_order = _SYNC_ORDER + _SCAL_ORDER
    sem_chunk = {c: nc.alloc_semaphore(f"sem_in_{c}") for c in chunk_order}
    keep_wait = bool(os.environ.get("KEEP_OUT_WAIT"))
    sem_out = nc.alloc_semaphore("sem_out")

    # per-tile bookkeeping for sync
    jobidx = {}
    counts = {"V": 0, "A": 0}
    for i, t in enumerate(tiles):
        counts[t["eng"]] += 1
        jobidx[i] = counts[t["eng"]]  # 1-based within its engine

    def emit_chunk_dma(eng, c):
        if c.startswith("a"):
            # a0: partitions 0-35 of gang 0; a0b: partitions 36-127 of
            # gang 0; a1: all partitions of gangs > 0
            p0, p1 = {"a0": (0, 36), "a0b": (36, P), "a1": (0, P)}[c]
            lo, hi = (0, P) if c != "a1" else (P, awidth)
            eng.dma_start(a_sb.ap()[p0:p1, lo:hi],
                          a_dram.ap()[p0:p1, lo:hi]).then_inc(sem_chunk[c], 16)
            return
        q, c0, c1 = _b_ranges(bcols)[c]
        eng.dma_start(b_sb.ap()[32 * q:32 * q + 4, c0:c1],
                      b_dram.ap()[:, c0:c1]).then_inc(sem_chunk[c], 16)

    with nc.Block("k") as blk:

        @blk.sync
        def _(eng):
            for c in _SYNC_ORDER:
                emit_chunk_dma(eng, c)
            # output from the idle sync engine.  Both consumer sems fire
            # AFTER their accumulator-column writes (the A-side then_inc
            # lands on ACTIVATION_READ_ACCUMULATOR, which does the write).
            # No completion wait: the NEFF postamble outlasts the drain.
            eng.wait_ge(sem_v, nV)
            eng.wait_ge(sem_a, nA)
            eng.dma_start(acc_dram.ap()[:, :], acc_sb.ap()[:, :]).then_inc(
                sem_out, 16)
            if keep_wait:
                eng.wait_ge(sem_out, 16)

        @blk.scalar
        def _(eng):
            for c in _SCAL_ORDER:
                emit_chunk_dma(eng, c)
            for i, t in enumerate(tiles):
                if t["eng"] != "A":
                    continue
                buf = i % 4
                eng.wait_ge(sem_mm, i + 1)
                red = ps.ap()[:, 2 * buf:2 * buf + t["nbank"], 0:t["width"]]
                eng.activation(
                    red, red,
                    mybir.ActivationFunctionType.Abs,
                    accum_out=acc_sb.ap()[:, i:i + 1],
                ).then_inc(sem_a, 1)

        @blk.tensor
        def _(eng):
            waited = set()
            for i, t in enumerate(tiles):
                for c in chunk_order:
                    if c in t["chunks"] and c not in waited:
                        waited.add(c)
                        eng.wait_ge(sem_chunk[c], 16)
                if i >= 4:
                    p = i - 4  # previous occupant of this 2-bank buffer
                    eng.wait_ge(sem_v if tiles[p]["eng"] == "V" else sem_a,
                                jobidx[p])
                buf = i % 4
                last = len(t["mms"]) - 1
                for j, (q, coff, n, bank, c0) in enumerate(t["mms"]):
                    ins = nc.tensor.matmul(
                        ps.ap()[:, 2 * buf + bank, c0:c0 + n],
                        a_sb.ap()[32 * q:32 * q + 4, t["acol"]:t["acol"] + P],
                        b_sb.ap()[32 * q:32 * q + 4, coff:coff + n],
                        start=True,
                        stop=True,
                        tile_position=(32 * q, 0),
                    )
                    if j == last:
                        ins.then_inc(sem_mm, 1)

        @blk.vector
        def _(eng):
            for i, t in enumerate(tiles):
                if t["eng"] != "V":
                    continue
                buf = i % 4
                eng.wait_ge(sem_mm, i + 1)
                eng.tensor_reduce(
                    acc_sb.ap()[:, i:i + 1],
                    ps.ap()[:, 2 * buf:2 * buf + t["nbank"], 0:t["width"]],
                    axis=mybir.AxisListType.XY, op=mybir.AluOpType.add,
                    apply_absolute_value=True,
                ).then_inc(sem_v, 1)

    nc.compile()
    return nc


def _get_nc(plan):
    key = ("nc", plan["nb"])
    if key not in _cache:
        _cache[key] = _build(plan)
    return _cache[key]


def _prepare(pred, gt, ift, imf):
    """Compact + pad + build per-core input maps."""
    import ml_dtypes

    p_full = np.asarray(pred).astype(np.float32)
    gt = np.asarray(gt).astype(np.float32)
    t_full = gt[:, ift]
    f_full = gt[:, imf] == 1
    idx = np.flatnonzero(f_full)
    n1 = len(idx)

    plan = _plan(n1)
    ring = plan["nb"] * P               # cyclic pair-cover modulus
    npad = NCORE * plan["nbc"] * P      # slot padding (zero A strips)
    p = np.zeros(npad, np.float32)
    t = np.zeros(npad, np.float32)
    w = np.zeros(npad, np.float32)
    p[:n1] = p_full[idx]
    t[:n1] = t_full[idx]
    w[:n1] = 1.0
    u = p * t

    # compaction makes the flags trivial: real entries are all flagged,
    # padded entries are exactly zero in every factor.
    A = np.ascontiguousarray(
        np.stack([u, w, p, t]).astype(ml_dtypes.bfloat16)
    )
    Bm = np.ascontiguousarray(
        np.stack([w, u, -t, -p]).astype(ml_dtypes.bfloat16)
    )

    nbc, eh, mainw = plan["nbc"], plan["eh"], plan["mainw"]
    gangs = plan["gangs"]
    awidth = P * len(gangs)
    in_maps = []
    for k in range(NCORE):
        a_rows = np.zeros((P, awidth), dtype=ml_dtypes.bfloat16)
        for g, sz in enumerate(gangs):
            off = sum(gangs[:g])
            for q in range(sz):
                blk = nbc * k + off + q
                a_rows[32 * q:32 * q + 4, P * g:P * g + P] = \
                    A[:, P * blk:P * blk + P]

        cols = (P * (nbc * k + 1) + np.arange(mainw)) % ring
        b_colsk = np.ascontiguousarray(Bm[:, cols])
        in_maps.append(
            {"a_rows": a_rows, "b_cols": np.ascontiguousarray(b_colsk)}
        )
    return in_maps, A, Bm, n1, plan


def kernel(pred, gt, gt_fracTime, gt_ifMOF):
    from concourse import bass_utils

    ift = int(np.asarray(gt_fracTime))
    imf = int(np.asarray(gt_ifMOF))

    in_maps, A, Bm, n1, plan = _prepare(pred, gt, ift, imf)
    nc = _get_nc(plan)
    res = bass_utils.run_bass_kernel_spmd(nc, in_maps,
                                          core_ids=list(range(NCORE)))

    # T = sum_{i<j} |M| (all device accumulator columns are weight 1)
    T = 0.0
    for r in res.results:
        T += r["acc"].astype(np.float64).sum()

    A64 = A.astype(np.float64)
    B64 = Bm.astype(np.float64)

    # the e=0 diagonal blocks (~2% of pairs) are summed here in fp64 over
    # the same bf16 values the device uses for everything else
    for b in range(plan["nb"]):
        sl = slice(P * b, P * (b + 1))
        G = A64[:, sl].T @ B64[:, sl]
        T += np.abs(np.triu(G, 1)).sum()

    # host closed form in fp64:
    # sum_{i<j} M = (sum_{i,j} M - sum_diag M) / 2
    S_all = (A64.sum(axis=1) * B64.sum(axis=1)).sum()
    D_diag = (A64 * B64).sum()
    S_half = (S_all - D_diag) / 2.0

    n_pairs = (float(n1) * float(n1) - float(n1)) / 2.0

    loss = 0.5 * (S_half + T) / 100.0 / n_pairs
    return np.asarray(np.float32(loss))


# revision 56
# speedup vs baseline: 1.0039x; 1.0039x over previous
"""Trainium2 Bass kernel for the pairwise concordance-index loss.

reference:
    loss = sum_{i<j, f_i=f_j=1} relu((p_i-p_j)(t_i-t_j)) / 100 / n_pairs

Math:
  Only flagged (f=1) entries contribute, so the host first COMPACTS the
  arrays to the n1 flagged entries (padded with zeros to NB*128), which
  shrinks the pairwise matrix from B^2 to ~(0.7B)^2.
  M[i,j] = (p_i-p_j)(t_i-t_j) = A^T B, rank 4:
      A = [u, 1, p, t],  B = [1, u, -t, -p],  u = p*t   (zeros in padding)
  sum relu(M) = 0.5*(sum M + sum |M|); sum M has an O(n) closed form done
  on the host in fp64; sum |M| is the O(n^2) part done on device.

Device decomposition (8 cores, identical program, data-sharded):
  The cyclic pair-cover ring has NB row-blocks of 128 rows, NB ODD and
  minimal (surplus block-SLOTS beyond the ring get all-zero A strips
  and contribute nothing).  For an odd ring, offsets e=1..(NB-1)/2
  cover every unordered block pair exactly once at full weight — no
  antipodal fold, no half-weight columns.  The e=0 diagonal blocks
  (~2% of pairs) are summed on the host in fp64.  Core k owns NBC
  block-slots as a quad gang (4) + duo gang (NBC-4).

Device structure (raw Bass, hand-rolled semaphores — no TileContext):
  PE: per 2-bank PSUM tile, K=4 bf16 matmuls packed into disjoint
  32-row PE groups via tile_position (concurrent matmuls always target
  distinct PSUM banks).  Each tile is consumed by ONE fused abs-row-sum
  job on the DVE (tensor_reduce(apply_absolute_value)) or the ScalarE
  (activation(Abs, accum_out)), greedily load-balanced (measured
  effective periods ~1131ns vs ~1181+339ns per 1024 elements).  PSUM
  is an 8-bank ring of four 2-bank buffers; the PE waits on the
  consumer semaphore before reusing a buffer.  The very first slot is
  emitted as two single-bank tiles so the PE can start on the minimum
  set of input chunks.

  Input DMAs: per-chunk semaphores (completions interleave across a
  queue's DMA engines, so cumulative counts are NOT safe).  The 'a'
  (weights) transfer is split per gang; a0 and b0h1 lead the sync
  queue while b1h1 leads the scalar queue, so tile 0's three chunks
  arrive on two queues in parallel.  Tiles are issued in data-arrival
  order.

  The output DMA is issued from the otherwise-idle sync engine, gated
  on both consumer sems (the A-side then_inc lands on
  ACTIVATION_READ_ACCUMULATOR, i.e. after the accumulator write), and
  its completion is NOT waited on: the NRT-injected postamble that
  follows (a ~7us storm resetting all 256 semaphores, engine by
  engine) far outlasts the DMA drain, so the store lands well before
  the runtime reads outputs.  Set KEEP_OUT_WAIT=1 to restore the
  explicit completion wait.

Known remaining lever (not yet implemented): the A strips for PE
  position q live at the same partitions (32q..32q+3) as B replica q,
  so the host could prepend the two 128-col strips to each replica's
  slab (b_dram becomes [16, 256+bcols], one row-block per replica) and
  drop the separate a0/a0b/a1 transfers entirely — 3 fewer sync-queue
  DMA instructions (~2us of issue time), and tile 0 would gate on a
  single chunk (~-1us on the first matmul).  Requires shifting all
  window offsets by +256 and re-deriving the chunk ranges.
"""

import numpy as np

B = 8192
P = 128
NCORE = 8
CUT = 2560

_cache = {}


def _plan(n1):
    """Compile-time plan derived from the flagged count.

    The cyclic pair-cover ring has nb blocks with nb ODD and minimal: for
    an odd ring, offsets e=1..(nb-1)/2 cover every unordered block pair
    exactly once (pair distance d>eh is covered by the other endpoint at
    e=nb-d), so there is no antipodal fold and no half-weight columns.
    The e=0 diagonal blocks are summed on the host.  Each core runs nbc
    block-SLOTS (identical SPMD program); nbc*NCORE may exceed nb — the
    surplus slots get all-zero A strips and contribute nothing.
    """
    nb = max(3, -(-n1 // P))        # 128-row blocks needed
    nb += 1 - (nb & 1)              # odd ring
    nbc = -(-nb // NCORE)           # block-slots per core
    eh = (nb - 1) // 2              # max offset (full weight)
    mainw = P * (nbc - 1 + eh)      # shared slab for e=1..eh
    gangs = [4] * (nbc // 4) + ([nbc % 4] if nbc % 4 else [])
    w = P * eh                      # main cols per block
    nfull, rem = w // 512, w % 512
    return dict(nb=nb, nbc=nbc, eh=eh, mainw=mainw,
                bcols=mainw, gangs=gangs, nfull=nfull, rem=rem)


# DMA queue orders.  Completions within a HWDGE queue interleave across
# physical DMA engines, so each chunk gets its OWN semaphore (wait >= 16
# means that chunk fully landed); the order below only sets arrival time.
# 'a0' (first gang's weights) and 'b0h1a' (tile 0's columns, small) lead
# parallel queues so the first matmul can start as early as possible.
_SPLIT0 = 1536
_SYNC_ORDER = ["a0", "b0h1a", "b0h1b", "a0b", "a1", "b2h1", "b0h2", "b2h2",
               "b3h2"]
_SCAL_ORDER = ["b1h1", "b3h1", "b1h2"]


def _b_ranges(bcols):
    """chunk name -> (replica, lo, hi) for the B column transfers."""
    r = {}
    for q in range(4):
        h1 = [(f"b{q}h1", q, 0, CUT)]
        if q == 0:
            h1 = [("b0h1a", 0, 0, _SPLIT0), ("b0h1b", 0, _SPLIT0, CUT)]
        for name, qq, lo, hi in h1:
            r[name] = (qq, lo, hi)
        r[f"b{q}h2"] = (q, CUT, bcols)
    return r


def _chunk_info():
    info = {}
    for i, c in enumerate(_SYNC_ORDER):
        info[c] = ("s", i + 0.0)
    for i, c in enumerate(_SCAL_ORDER):
        info[c] = ("c", i + 0.5)
    return info


def _layout(plan):
    """Ordered tile descriptors: matmul lists, reduce specs, DMA gates."""
    gangs, rem, mainw = plan["gangs"], plan["rem"], plan["mainw"]
    cinfo = _chunk_info()
    tiles = []
    for g, sz in enumerate(gangs):
        off = sum(gangs[:g])
        npair = (sz + 1) // 2
        for pi in range(npair):
            qs = [q for q in (2 * pi, 2 * pi + 1) if q < sz]
            for s in range(plan["nfull"]):
                if g == 0 and pi == 0 and s == 0:
                    # very first slot as two single-bank tiles: tile 0 only
                    # needs the sync queue's first two chunks, so the PE
                    # starts ~0.7us earlier; consumers are idle then, so
                    # the extra small job costs no makespan
                    for q in qs:
                        mms = [(q, P * (off + q), 512, 0, 0)]
                        tiles.append(dict(acol=P * g, mms=mms, kind="main"))
                    continue
                mms = [(q, P * (off + q) + 512 * s, 512, qi, 0)
                       for qi, q in enumerate(qs)]
                tiles.append(dict(acol=P * g, mms=mms, kind="main"))
    # remainder tiles: the last rem columns of each block's main span
    if rem:
        for g, sz in enumerate(gangs):
            off = sum(gangs[:g])
            for pi in range((sz + 1) // 2):
                qs = [q for q in (2 * pi, 2 * pi + 1) if q < sz]
                mms = [(q, P * (off + q) + 512 * plan["nfull"], rem, qi, 0)
                       for qi, q in enumerate(qs)]
                tiles.append(dict(acol=P * g, mms=mms, kind="rem"))
    # per-tile chunk requirements -> (queue, threshold); arrival rank
    branges = _b_ranges(plan["bcols"])
    for t in tiles:
        # gang-0 weights arrive in two pieces: a0 = partitions 0-35 (q0/q1
        # strips, 36 descriptors — fast), a0b = partitions 36-127 (q2/q3)
        if t["acol"] == 0:
            need = {"a0" if q < 2 else "a0b" for (q, *_r) in t["mms"]}
        else:
            need = {"a1"}
        for (q, coff, n, _b, _c) in t["mms"]:
            for name, (qq, lo, hi) in branges.items():
                if qq == q and coff < hi and coff + n > lo:
                    need.add(name)
        t["nbank"] = len(set(b for (_, _, _, b, _) in t["mms"]))
        t["width"] = max(c + n for (_, _, n, _, c) in t["mms"])
        t["fd"] = sum(n for (_, _, n, _, _) in t["mms"])
        t["chunks"] = need
        t["rank"] = max(cinfo[c][1] for c in need)
    # issue order: by data arrival, stable within rank
    tiles.sort(key=lambda t: t["rank"])
    # consumer assignment: greedy balance of projected finish times using
    # HW-measured effective periods (V 1131ns @ fd=1024, A 1181ns), then
    # pairwise V<->A swaps (count-preserving, so the ring alternation
    # pattern is kept) to close any residual finish-time gap
    def cv(fd):
        return (fd + 62) / 0.96

    def ca(fd):
        return fd / 1.2 + 328.0

    tv = ta = 0.0
    for t in tiles:
        if tv + cv(t["fd"]) <= ta + ca(t["fd"]):
            t["eng"] = "V"
            tv += cv(t["fd"])
        else:
            t["eng"] = "A"
            ta += ca(t["fd"])

    def makespan():
        return max(sum(cv(t["fd"]) for t in tiles if t["eng"] == "V"),
                   sum(ca(t["fd"]) for t in tiles if t["eng"] == "A"))

    best = makespan()
    improved = True
    while improved:
        improved = False
        for t1 in tiles:
            for t2 in tiles:
                if t1["eng"] == "V" and t2["eng"] == "A":
                    t1["eng"], t2["eng"] = "A", "V"
                    m = makespan()
                    if m < best - 1e-9:
                        best = m
                        improved = True
                    else:
                        t1["eng"], t2["eng"] = "V", "A"
    return tiles


def _build(plan):
    """Build + compile the raw Bass module (once per plan)."""
    import os
    import concourse.bacc as bacc
    import concourse.mybir as mybir

    f32 = mybir.dt.float32
    bf16 = mybir.dt.bfloat16
    nc = bacc.Bacc("TRN2", target_bir_lowering=False, debug=False,
                   num_devices=NCORE)

    gangs = plan["gangs"]
    bcols = plan["bcols"]
    awidth = P * len(gangs)
    tiles = _layout(plan)
    njobs = len(tiles)
    nV = sum(1 for t in tiles if t["eng"] == "V")
    nA = njobs - nV

    a_dram = nc.dram_tensor("a_rows", [P, awidth], bf16, kind="ExternalInput")
    b_dram = nc.dram_tensor("b_cols", [4, bcols], bf16, kind="ExternalInput")
    acc_dram = nc.dram_tensor("acc", [P, njobs], f32, kind="ExternalOutput")

    a_sb = nc.alloc_sbuf_tensor("a_sb", [P, awidth], bf16)
    b_sb = nc.alloc_sbuf_tensor("b_sb", [P, bcols], bf16)
    acc_sb = nc.alloc_sbuf_tensor("acc_sb", [P, njobs], f32)
    ps = nc.alloc_psum_tensor("ps", [P, 8, 512], f32)

    sem_mm = nc.alloc_semaphore("sem_mm")    # tiles filled by PE
    sem_v = nc.alloc_semaphore("sem_v")      # DVE jobs done
    sem_a = nc.alloc_semaphore("sem_a")      # ACT jobs done
    # one sem per input chunk: completions interleave within a queue
    chunk_order = _SYNC_ORDER + _SCAL_ORDER
    sem_chunk = {c: nc.alloc_semaphore(f"sem_in_{c}") for c in chunk_order}
    keep_wait = bool(os.environ.get("KEEP_OUT_WAIT"))
    sem_out = nc.alloc_semaphore("sem_out")

    # per-tile bookkeeping for sync
    jobidx = {}
    counts = {"V": 0, "A": 0}
    for i, t in enumerate(tiles):
        counts[t["eng"]] += 1
        jobidx[i] = counts[t["eng"]]  # 1-based within its engine

    def emit_chunk_dma(eng, c):
        if c.startswith("a"):
            # a0: partitions 0-35 of gang 0; a0b: partitions 36-127 of
            # gang 0; a1: all partitions of gangs > 0
            p0, p1 = {"a0": (0, 36), "a0b": (36, P), "a1": (0, P)}[c]
            lo, hi = (0, P) if c != "a1" else (P, awidth)
            eng.dma_start(a_sb.ap()[p0:p1, lo:hi],
                          a_dram.ap()[p0:p1, lo:hi]).then_inc(sem_chunk[c], 16)
            return
        q, c0, c1 = _b_ranges(bcols)[c]
        eng.dma_start(b_sb.ap()[32 * q:32 * q + 4, c0:c1],
                      b_dram.ap()[:, c0:c1]).then_inc(sem_chunk[c], 16)

    with nc.Block("k") as blk:

        @blk.sync
        def _(eng):
            for c in _SYNC_ORDER:
                emit_chunk_dma(eng, c)
            # output from the idle sync engine.  Both consumer sems fire
            # AFTER their accumulator-column writes (the A-side then_inc
            # lands on ACTIVATION_READ_ACCUMULATOR, which does the write).
            # No completion wait: the NEFF postamble outlasts the drain.
            eng.wait_ge(sem_v, nV)
            eng.wait_ge(sem_a, nA)
            eng.dma_start(acc_dram.ap()[:, :], acc_sb.ap()[:, :]).then_inc(
                sem_out, 16)
            if keep_wait:
                eng.wait_ge(sem_out, 16)

        @blk.scalar
        def _(eng):
            for c in _SCAL_ORDER:
                emit_chunk_dma(eng, c)
            for i, t in enumerate(tiles):
                if t["eng"] != "A":
                    continue
                buf = i % 4
                eng.wait_ge(sem_mm, i + 1)
                red = ps.ap()[:, 2 * buf:2 * buf + t["nbank"], 0:t["width"]]
                eng.activation(
                    red, red,
                    mybir.ActivationFunctionType.Abs,
                    accum_out=acc_sb.ap()[:, i:i + 1],
                ).then_inc(sem_a, 1)

        @blk.tensor
        def _(eng):
            waited = set()
            for i, t in enumerate(tiles):
                for c in chunk_order:
                    if c in t["chunks"] and c not in waited:
                        waited.add(c)
                        eng.wait_ge(sem_chunk[c], 16)
                if i >= 4:
                    p = i - 4  # previous occupant of this 2-bank buffer
                    eng.wait_ge(sem_v if tiles[p]["eng"] == "V" else sem_a,
                                jobidx[p])
                buf = i % 4
                last = len(t["mms"]) - 1
                for j, (q, coff, n, bank, c0) in enumerate(t["mms"]):
                    ins = nc.tensor.matmul(
                        ps.ap()[:, 2 * buf + bank, c0:c0 + n],
                        a_sb.ap()[32 * q:32 * q + 4, t["acol"]:t["acol"] + P],
                        b_sb.ap()[32 * q:32 * q + 4, coff:coff + n],
                        start=True,
                        stop=True,
                        tile_position=(32 * q, 0),
                    )
                    if j == last:
                        ins.then_inc(sem_mm, 1)

        @blk.vector
        def _(eng):
            for i, t in enumerate(tiles):
                if t["eng"] != "V":
                    continue
                buf = i % 4
                eng.wait_ge(sem_mm, i + 1)
                eng.tensor_reduce(
                    acc_sb.ap()[:, i:i + 1],
                    ps.ap()[:, 2 * buf:2 * buf + t["nbank"], 0:t["width"]],
                    axis=mybir.AxisListType.XY, op=mybir.AluOpType.add,
                    apply_absolute_value=True,
                ).then_inc(sem_v, 1)

    nc.compile()
    return nc


def _get_nc(plan):
    key = ("nc", plan["nb"])
    if key not in _cache:
        _cache[key] = _build(plan)
    return _cache[key]


def _prepare(pred, gt, ift, imf):
    """Compact + pad + build per-core input maps."""
    import ml_dtypes

    p_full = np.asarray(pred).astype(np.float32)
    gt = np.asarray(gt).astype(np.float32)
    t_full = gt[:, ift]
    f_full = gt[:, imf] == 1
    idx = np.flatnonzero(f_full)
    n1 = len(idx)

    plan = _plan(n1)
    ring = plan["nb"] * P               # cyclic pair-cover modulus
    npad = NCORE * plan["nbc"] * P      # slot padding (zero A strips)
    p = np.zeros(npad, np.float32)
    t = np.zeros(npad, np.float32)
    w = np.zeros(npad, np.float32)
    p[:n1] = p_full[idx]
    t[:n1] = t_full[idx]
    w[:n1] = 1.0
    u = p * t

    # compaction makes the flags trivial: real entries are all flagged,
    # padded entries are exactly zero in every factor.
    A = np.ascontiguousarray(
        np.stack([u, w, p, t]).astype(ml_dtypes.bfloat16)
    )
    Bm = np.ascontiguousarray(
        np.stack([w, u, -t, -p]).astype(ml_dtypes.bfloat16)
    )

    nbc, eh, mainw = plan["nbc"], plan["eh"], plan["mainw"]
    gangs = plan["gangs"]
    awidth = P * len(gangs)
    in_maps = []
    for k in range(NCORE):
        a_rows = np.zeros((P, awidth), dtype=ml_dtypes.bfloat16)
        for g, sz in enumerate(gangs):
            off = sum(gangs[:g])
            for q in range(sz):
                blk = nbc * k + off + q
                a_rows[32 * q:32 * q + 4, P * g:P * g + P] = \
                    A[:, P * blk:P * blk + P]

        cols = (P * (nbc * k + 1) + np.arange(mainw)) % ring
        b_colsk = np.ascontiguousarray(Bm[:, cols])
        in_maps.append(
            {"a_rows": a_rows, "b_cols": np.ascontiguousarray(b_colsk)}
        )
    return in_maps, A, Bm, n1, plan


def kernel(pred, gt, gt_fracTime, gt_ifMOF):
    from concourse import bass_utils

    ift = int(np.asarray(gt_fracTime))
    imf = int(np.asarray(gt_ifMOF))

    in_maps, A, Bm, n1, plan = _prepare(pred, gt, ift, imf)
    nc = _get_nc(plan)
    res = bass_utils.run_bass_kernel_spmd(nc, in_maps,
                                          core_ids=list(range(NCORE)))

    # T = sum_{i<j} |M| (all device accumulator columns are weight 1)
    T = 0.0
    for r in res.results:
        T += r["acc"].astype(np.float64).sum()

    A64 = A.astype(np.float64)
    B64 = Bm.astype(np.float64)

    # the e=0 diagonal blocks (~2% of pairs) are summed here in fp64 over
    # the same bf16 values the device uses for everything else
    for b in range(plan["nb"]):
        sl = slice(P * b, P * (b + 1))
        G = A64[:, sl].T @ B64[:, sl]
        T += np.abs(np.triu(G, 1)).sum()

    # host closed form in fp64:
    # sum_{i<j} M = (sum_{i,j} M - sum_diag M) / 2
    S_all = (A64.sum(axis=1) * B64.sum(axis=1)).sum()
    D_diag = (A64 * B64).sum()
    S_half = (S_all - D_diag) / 2.0

    n_pairs = (float(n1) * float(n1) - float(n1)) / 2.0

    loss = 0.5 * (S_half + T) / 100.0 / n_pairs
    return np.asarray(np.float32(loss))
